# revision 21
# baseline (speedup 1.0000x reference)
"""Causal self-attention kernel for Trainium2, 8 NeuronCores.

Sharding: DP4 x TP2. Core c = 2*b + g handles batch b (2048 tokens) and
head-group g (8 of 16 heads). Per core:
  - x arrives pre-cast to bf16 AND pre-transposed on the host (d_model
    rows), so it loads with plain DMAs (no xbar transpose),
  - QKV matmuls in bf16: Q,K dim-major ([head_dim, tokens]), V token-major
    with a ones column at col 64 (softmax denominator for free),
  - attention per head pair: scores^T = K_h^T-tile @ Q_h in [k, q] layout
    with both heads' QK matmuls in different PE row groups (concurrent),
    one wide exp on ACT (1/sqrt(64) folded into its scale) into bf16 probs,
    causal handling by skipping fully-masked column ranges in both the
    exp and the PV matmul, and a 0/1 mask multiply on the 128-wide
    diagonal band,
  - normalization via reciprocal_approx_fast (through an SBUF copy of
    the PSUM ones-row; the custom DVE op misreads PSUM directly) +
    gpsimd partition_broadcast, y stored dim-major in bf16,
  - pairwise AllGather (cores 2b, 2b+1) of the bf16 y halves, then a
    bf16 projection with FULL contraction (1024) over HALF the output
    columns (512 per core), DMA'd straight to the output tensor. The
    host concatenates the two column halves.

QKV work for token tile n+1 and projection for tile n-1 are emitted
interleaved with attention for tile n (attention is exp-cadence-bound,
so the PE needs independent matmuls while ACT drains); chunk 3's K/V
chains are hand-placed inside attention(3) just before their first
consumers. Dummy matmuls on the mask tile bridge the startup DMA wait
and the final AllGather window so HAM keeps the PE at 2.4 GHz.

Everything (shapes, sharding) is hardcoded for
x: [4, 2048, 1024], w_qkv: [1024, 3072], w_proj: [1024, 1024], f32.
"""

import ml_dtypes
import numpy as np

import concourse.bacc as bacc
import concourse.mybir as mybir
import concourse.tile as tile
from concourse.bass_utils import run_bass_kernel_spmd

F32 = mybir.dt.float32
BF16 = mybir.dt.bfloat16

S = 2048  # tokens per core (one batch element)
D = 1024  # d_model
HL = 8  # heads per core (local)
HD = 64  # head dim
GD = HL * HD  # 512, head-group dim
NQT = S // 512  # 4 q-tiles of 512
NDM = D // 128  # 8 d_model chunks
NTOK = S // 128  # 16 token tiles of 128
VW = 72  # v_sb column width (64 dims + ones col @64 + pad)

_NC_CACHE = {}
_DEBUG = False


def _qkv_units(nc, P, n):
    """QKV matmul chains for token tile n, as separately emittable units."""
    units = []

    def qk_chain(m):
        def emit():
            ps = P.b1_ps.tile([128, 512], F32, tag="b1", name="qkps")
            for k in range(NDM):
                nc.tensor.matmul(
                    ps,
                    P.w_sb[:, k, m * 128 : (m + 1) * 128],
                    P.xT[:, k, n * 512 : (n + 1) * 512],
                    start=(k == 0),
                    stop=(k == NDM - 1),
                )
            nc.vector.tensor_copy(
                out=P.qkT[:, m, n * 512 : (n + 1) * 512], in_=ps
            )

        return emit

    def v_chain(t4):
        def emit():
            t = n * 4 + t4
            ps = P.b1_ps.tile([128, 512], F32, tag="b1", name="vps")
            for k in range(NDM):
                nc.tensor.matmul(
                    ps,
                    P.xT[:, k, t * 128 : (t + 1) * 128],
                    P.w_sb[:, k, 2 * GD : 3 * GD],
                    start=(k == 0),
                    stop=(k == NDM - 1),
                )
            nc.vector.tensor_copy(
                out=P.v_sb[:, t, :, 0:HD],
                in_=ps.rearrange("p (h d) -> p h d", h=HL),
            )

        return emit

    for m in range(2 * GD // 128):
        units.append(qk_chain(m))
    for t4 in range(4):
        units.append(v_chain(t4))
    return units


def _attn_units(nc, P, j):
    """Attention units for q-tile j: per head pair, c-groups + epilogue."""
    units = []
    for hp in range(HL // 2):
        yps = {}

        def alloc(hp=hp, yps=yps):
            for hi in range(2):
                yps[hi] = P.y_ps.tile(
                    [128, 512], F32, tag=f"yps{hi}", name=f"yps{hi}", bufs=1
                )

        units.append(alloc)

        def cgroup(c, hp=hp, yps=yps):
            def emit():
                d = c - 4 * j  # >= 0 on the diagonal band
                off = max(d, 0) * 128  # columns below off are fully masked
                sps2 = P.attn_ps.tile(
                    [128, 2, 512], F32, tag="sps2", name="sps2"
                )
                for hi in range(2):
                    h = 2 * hp + hi
                    po = (h % 2) * 64
                    nc.tensor.matmul(
                        sps2[:, hi, off:512],
                        P.qkT[po : po + 64, 4 + h // 2, c * 128 : (c + 1) * 128],
                        P.qkT[po : po + 64, h // 2, j * 512 + off : (j + 1) * 512],
                        start=True,
                        stop=True,
                    )
                probs2 = P.probs_p.tile(
                    [128, 2, 512], BF16, tag="probs", name="probs"
                )
                nc.scalar.activation(
                    out=probs2[:, :, off:512],
                    in_=sps2[:, :, off:512],
                    func=mybir.ActivationFunctionType.Exp,
                    scale=0.125,
                )
                if d >= 0:
                    for hi in range(2):
                        nc.vector.tensor_mul(
                            probs2[:, hi, off : off + 128],
                            probs2[:, hi, off : off + 128],
                            P.mask_sb,
                        )
                for hi in range(2):
                    h = 2 * hp + hi
                    nc.tensor.matmul(
                        yps[hi][0:VW, off:512],
                        P.v_sb[:, c, h, :],
                        probs2[:, hi, off:512],
                        start=(c == 0),
                        stop=(c == 4 * j + 3),
                    )

            return emit

        for c in range(4 * j + 4):
            units.append(cgroup(c))

        def epilogue(hp=hp, yps=yps):
            # fast reciprocal of the PSUM ones-row, partition broadcast,
            # scale y into dim-major bf16 yT, then immediately ship this
            # head-pair's yT rows to the collective staging buffer so the
            # AllGather's inputs trickle in as head pairs finish.
            for hi in range(2):
                h = 2 * hp + hi
                po = (h % 2) * 64
                den0 = P.den_p.tile([1, 512], F32, tag="den0", name="den0")
                nc.vector.tensor_copy(out=den0, in_=yps[hi][HD : HD + 1, :])
                den = P.den_p.tile([1, 512], F32, tag="den", name="den")
                nc.vector.reciprocal_approx_fast(out=den, in_=den0)
                denb = P.den_p.tile([HD, 512], F32, tag="denb", name="denb")
                nc.gpsimd.partition_broadcast(denb, den)
                nc.vector.tensor_mul(
                    P.yT[po : po + 64, h // 2, j * 512 : (j + 1) * 512],
                    yps[hi][0:HD, :],
                    denb,
                )
            nc.sync.dma_start(
                out=P.cc_in[j, hp * 128 : (hp + 1) * 128, :],
                in_=P.yT[:, hp, j * 512 : (j + 1) * 512],
            )

        units.append(epilogue)
    return units


def _proj_units(nc, P, j):
    """Projection for q-tile j: full 1024 contraction from the AllGathered
    yTf, half (512) output columns, token-major, DMA'd straight to out."""
    units = []
    for mt in range(4 * j, 4 * j + 4):
        def emit(mt=mt):
            ps = P.b1_ps.tile([128, 512], F32, tag="b1", name="ops")
            for kk in range(NDM):
                nc.tensor.matmul(
                    ps,
                    P.yTf[:, kk, mt * 128 : (mt + 1) * 128],
                    P.wp_sb[:, kk, :],
                    start=(kk == 0),
                    stop=(kk == NDM - 1),
                )
            osb = P.out_p.tile([128, 512], F32, tag="osb", name="osb")
            if j == NQT - 1:
                nc.scalar.copy(out=osb, in_=ps)
            else:
                nc.vector.tensor_copy(out=osb, in_=ps)
            nc.sync.dma_start(
                out=P.out[mt * 128 : (mt + 1) * 128, :], in_=osb
            )

        units.append(emit)
    return units


def _ag_chunk(nc, P, j):
    """Pairwise AllGather of chunk j's bf16 y half (inputs staged into
    cc_in by the epilogues as head pairs finish), then DMA both group
    halves back into yTf, split per 256-token half so the first
    projection tiles can start before the whole chunk lands."""
    lo = j * 512
    nc.gpsimd.collective_compute(
        "AllGather",
        mybir.AluOpType.bypass,
        replica_groups=[[0, 1], [2, 3], [4, 5], [6, 7]],
        ins=[P.cc_in[j].opt()],
        outs=[P.cc_out[j].opt()],
    )
    for h2 in range(2):
        for gg in range(2):
            for kk in range(GD // 128):
                nc.sync.dma_start(
                    out=P.yTf[
                        :, gg * 4 + kk, lo + h2 * 256 : lo + (h2 + 1) * 256
                    ],
                    in_=P.cc_out[
                        j,
                        gg,
                        kk * 128 : (kk + 1) * 128,
                        h2 * 256 : (h2 + 1) * 256,
                    ],
                )


class _Ctx:
    pass


def _interleave(a_units, *extra):
    """Merge extra unit lists evenly into the (longer) attention list."""
    merged = []
    ex = [u for lst in extra for u in lst]
    k = 0
    for i, u in enumerate(a_units):
        merged.append(u)
        while k < len(ex) and k * len(a_units) < (i + 1) * len(ex):
            merged.append(ex[k])
            k += 1
    merged.extend(ex[k:])
    return merged


def _build_nc():
    nc = bacc.Bacc(None, num_devices=8)
    P = _Ctx()

    xTin = nc.dram_tensor("xT", [D, S], BF16, kind="ExternalInput").ap()
    wqkv = nc.dram_tensor("wqkv", [D, 3 * GD], BF16, kind="ExternalInput").ap()
    wproj = nc.dram_tensor("wproj", [D, 512], BF16, kind="ExternalInput").ap()
    masks = nc.dram_tensor("masks", [128, 128], BF16, kind="ExternalInput").ap()
    P.out = nc.dram_tensor("out", [S, 512], F32, kind="ExternalOutput").ap()

    with tile.TileContext(nc) as tc:
        with (
            tc.tile_pool(name="const", bufs=1) as const,
            tc.tile_pool(name="w_p", bufs=1) as w_p,
            tc.tile_pool(name="big_p", bufs=1) as big_p,
            tc.tile_pool(name="probs_p", bufs=8) as probs_p,
            tc.tile_pool(name="den_p", bufs=2) as den_p,
            tc.tile_pool(name="out_p", bufs=2) as out_p,
            tc.tile_pool(name="b1_ps", bufs=2, space="PSUM") as b1_ps,
            tc.tile_pool(name="attn_ps", bufs=2, space="PSUM") as attn_ps,
            tc.tile_pool(name="y_ps", bufs=1, space="PSUM") as y_ps,
            tc.tile_pool(name="dram", bufs=1, space="DRAM") as dram,
        ):
            P.probs_p, P.den_p, P.out_p = probs_p, den_p, out_p
            P.b1_ps, P.attn_ps, P.y_ps = b1_ps, attn_ps, y_ps

            P.mask_sb = const.tile([128, 128], BF16, name="mask_sb")
            nc.sync.dma_start(out=P.mask_sb, in_=masks)

            # Preheat: dummy matmuls on the (tiny, early-landing) mask tile
            # keep the PE's HAM activity window busy across the input-DMA
            # wait so the first real matmuls run at 2.4 GHz, not 1.2.
            php = P.b1_ps.tile([128, 512], F32, tag="b1", name="php")
            for ph in range(160):
                nc.tensor.matmul(
                    php[:, 0:128], P.mask_sb, P.mask_sb, start=True, stop=True
                )

            # Startup DMA order follows first-use order: x token-chunk 0 and
            # the Q-block of w_qkv gate the very first QKV chain, the K/V
            # blocks and later token chunks stream in behind them.
            P.w_sb = w_p.tile([128, NDM, 3 * GD], BF16, name="w_sb")
            P.wp_sb = w_p.tile([128, NDM, 512], BF16, name="wp_sb")
            P.xT = big_p.tile([128, NDM, S], BF16, name="xT")

            def x_chunk(tc4):
                for e in range(NDM):
                    nc.sync.dma_start(
                        out=P.xT[:, e, tc4 * 512 : (tc4 + 1) * 512],
                        in_=xTin[
                            e * 128 : (e + 1) * 128,
                            tc4 * 512 : (tc4 + 1) * 512,
                        ],
                    )

            def w_block(blk):
                for k in range(NDM):
                    nc.sync.dma_start(
                        out=P.w_sb[:, k, blk * GD : (blk + 1) * GD],
                        in_=wqkv[
                            k * 128 : (k + 1) * 128,
                            blk * GD : (blk + 1) * GD,
                        ],
                    )

            for k in range(NDM):  # interleave x chunk 0 with Q weights
                nc.sync.dma_start(
                    out=P.xT[:, k, 0:512], in_=xTin[k * 128 : (k + 1) * 128, 0:512]
                )
                nc.sync.dma_start(
                    out=P.w_sb[:, k, 0:GD],
                    in_=wqkv[k * 128 : (k + 1) * 128, 0:GD],
                )
            w_block(1)  # K
            x_chunk(1)
            w_block(2)  # V
            x_chunk(2)
            x_chunk(3)
            for kk in range(NDM):
                nc.sync.dma_start(
                    out=P.wp_sb[:, kk, :],
                    in_=wproj[kk * 128 : (kk + 1) * 128, :],
                )

            P.qkT = big_p.tile([128, 2 * GD // 128, S], BF16, name="qkT")
            P.v_sb = big_p.tile([128, NTOK, HL, VW], BF16, name="v_sb")
            nc.vector.memset(P.v_sb[:, :, :, HD : HD + 1], 1.0)
            P.yT = big_p.tile([128, GD // 128, S], BF16, name="yT")
            P.yTf = big_p.tile([128, NDM, S], BF16, name="yTf")

            P.cc_in = dram.tile([NQT, GD, 512], BF16, name="cc_in")
            P.cc_out = dram.tile([NQT, 2, GD, 512], BF16, name="cc_out")

            # Filler rebalance: attention(n) is exp-cadence-bound, with PE
            # slack growing with n (more cgroups). qkv(n+1) fills n=0..1;
            # chunk 3's Q chains fill attention(2) and its K/V chains fill
            # the front of attention(3) (their consumers come late there).
            for u in _qkv_units(nc, P, 0):
                u()
            for n in range(NQT):
                a_units = _attn_units(nc, P, n)
                if n < 2:
                    q_units = _qkv_units(nc, P, n + 1)
                elif n == 2:
                    q_units = _qkv_units(nc, P, 3)[0:4]  # Q chains
                else:
                    q_units = P.qkv3_rest  # K + V chains
                p_units = _proj_units(nc, P, n - 1) if n >= 1 else []
                if n == 2:
                    P.qkv3_rest = _qkv_units(nc, P, 3)[4:12]
                if n == 3:
                    # Hand-placed: each chunk-3 K/V chain must be EMITTED
                    # before its first consumer (program order is dataflow
                    # order). hp0 consumes c=12..15 at units 13..16, so
                    # m4/v12..v15 go into its c0..c11 zone; m5/m6/m7 are
                    # only needed by hp1/2/3 (units 31/49/67).
                    m4, m5, m6, m7, v12, v13, v14, v15 = q_units
                    p0, p1, p2, p3 = p_units
                    ins = {
                        2: m4, 4: v12, 6: v13, 8: v14, 10: v15,
                        20: m5, 38: m6, 44: p0, 50: p1, 56: m7,
                        60: p2, 66: p3,
                    }
                    for i, u in enumerate(a_units):
                        u()
                        if i in ins:
                            ins[i]()
                else:
                    for u in _interleave(a_units, q_units, p_units):
                        u()
                _ag_chunk(nc, P, n)
            # Tail warm-keeper: dummy matmuls run while the last AllGather
            # is in flight so the final projection starts at 2.4 GHz.
            php2 = P.b1_ps.tile([128, 512], F32, tag="b1", name="php2")
            for ph in range(50):
                nc.tensor.matmul(
                    php2,
                    P.yT[:, 3, 1536:1664],
                    P.xT[:, 0, 0:512],
                    start=True,
                    stop=True,
                )
            for u in _proj_units(nc, P, NQT - 1):
                u()
            if _DEBUG:
                for nm, t in (
                    ("dbg_qkT", P.qkT),
                    ("dbg_yT", P.yT),
                    ("dbg_yTf", P.yTf),
                ):
                    dt_ = nc.dram_tensor(
                        nm, list(t.shape), BF16, kind="ExternalOutput"
                    ).ap()
                    nc.sync.dma_start(out=dt_, in_=t[:, :, :])

    nc.compile()
    return nc


def _host_consts():
    ki = np.arange(128)[:, None]
    qj = np.arange(128)[None, :]
    masks = (qj >= ki).astype(ml_dtypes.bfloat16)  # [128, 128] diagonal band
    return masks


def _in_maps(x, w_qkv, w_proj):
    masks = _host_consts()
    maps = []
    for c in range(8):
        b, g = c // 2, c % 2
        wq = w_qkv[:, g * GD : (g + 1) * GD]
        wk = w_qkv[:, D + g * GD : D + (g + 1) * GD]
        wv = w_qkv[:, 2 * D + g * GD : 2 * D + (g + 1) * GD]
        maps.append(
            {
                "xT": np.ascontiguousarray(x[b].T).astype(ml_dtypes.bfloat16),
                "wqkv": np.ascontiguousarray(
                    np.concatenate([wq, wk, wv], axis=1)
                ).astype(ml_dtypes.bfloat16),
                "wproj": np.ascontiguousarray(
                    w_proj[:, g * 512 : (g + 1) * 512]
                ).astype(ml_dtypes.bfloat16),
                "masks": masks,
            }
        )
    return maps


def kernel(x, w_qkv, w_proj):
    x = np.ascontiguousarray(x, dtype=np.float32)
    w_qkv = np.ascontiguousarray(w_qkv, dtype=np.float32)
    w_proj = np.ascontiguousarray(w_proj, dtype=np.float32)
    if "nc" not in _NC_CACHE:
        _NC_CACHE["nc"] = _build_nc()
    nc = _NC_CACHE["nc"]
    r = run_bass_kernel_spmd(nc, _in_maps(x, w_qkv, w_proj), list(range(8)))
    return np.stack(
        [
            np.concatenate(
                [r.results[2 * b]["out"], r.results[2 * b + 1]["out"]],
                axis=1,
            )
            for b in range(4)
        ],
        axis=0,
    )
